# revision 3
# baseline (speedup 1.0000x reference)
"""Trainium2 Bass kernel for nn_ComparisonLoss (per-class balanced BCE loss).

Strategy
--------
Data-parallel over the batch across 8 NeuronCores. The whole loss reduces to a
single streaming pass per core that produces 7 per-class sufficient statistics
(each a [40]-vector), followed by a tiny host-side epilogue:

  With t in {0,1}:  u = pred * (1 - 2t)  ==>  bce = softplus(u)
  and |sigmoid(pred) - t| < 0.1  <=>  bce < ln(10/9)   (easy bin)
      |sigmoid(pred) - t| >= 0.9 <=>  bce >= ln(10)    (hard bin)
  (softplus is monotonic, so bin tests become thresholds on bce itself).

Per-class sums accumulated on-device (via ones-vector matmuls into PSUM):
  0: sum(w0)          w0 = 1 - drop*hard   (pass-1 weights)
  1: sum(t*w0)        (pos_sum)
  2: sum(t)
  3: sum(bce*w0)
  4: sum(bce*w0*t)
  5: sum(bce*easy)    (w0 == 1 on easy elements since easy & hard are disjoint)
  6: sum(bce*easy*t)

The majority/minority masking + rescaling of the reference only needs these
sums; the final scalar mean is computed on host from the gathered [7,40]
partials. The 0/1-valued tensors (t, masks, w0) are exact in bf16, so all mask
math runs in bf16 (2x DVE tensor_tensor mode) and the count sums stay
integer-exact in fp32 PSUM, making the majority decisions match the reference
bit-for-bit.
"""

import sys

for _p in ("/opt/trn_rl_repo",):
    if _p not in sys.path:
        sys.path.insert(0, _p)

import numpy as np
import ml_dtypes

import concourse.bacc as bacc
import concourse.tile as tile
from concourse import mybir

# ---- problem constants (hardcoded; kernel.py must be self-contained) ----
B, C = 262144, 40
N_CORES = 8
ROWS_PER_CORE = B // N_CORES          # 32768
P = 128                               # SBUF partitions
ROWS_PER_PART = ROWS_PER_CORE // P    # 256 rows per partition per core
R_ST = 32                             # rows per partition per supertile
N_ST = ROWS_PER_PART // R_ST          # 8 supertiles
F = R_ST * C                          # 1280 free elems per partition per supertile
BLK = 320                             # matmul free width (multiple of C, <=512)
NBLK = F // BLK                       # 4
N_ACC = 7

C_EASY = float(np.log(10.0 / 9.0))    # softplus(-ln 9)
C_HARD = float(np.log(10.0))          # softplus(+ln 9)

F32 = mybir.dt.float32
BF16 = mybir.dt.bfloat16


def _build_bass():
    nc = bacc.Bacc("TRN2", target_bir_lowering=False, debug=False)

    pred = nc.dram_tensor("pred", [ROWS_PER_CORE, C], F32, kind="ExternalInput")
    tgt = nc.dram_tensor("target", [ROWS_PER_CORE, C], BF16, kind="ExternalInput")
    rnd = nc.dram_tensor("rand", [ROWS_PER_CORE, C], BF16, kind="ExternalInput")
    rate = nc.dram_tensor("rate", [P, F], BF16, kind="ExternalInput")
    out = nc.dram_tensor("out", [1, N_ACC * BLK], F32, kind="ExternalOutput")

    # row index = st*(P*R_ST) + p*R_ST + r  -> partition p holds contiguous rows
    pred_v = pred.rearrange("(s p r) c -> s p (r c)", s=N_ST, p=P, r=R_ST)
    tgt_v = tgt.rearrange("(s p r) c -> s p (r c)", s=N_ST, p=P, r=R_ST)
    rnd_v = rnd.rearrange("(s p r) c -> s p (r c)", s=N_ST, p=P, r=R_ST)

    TT = mybir.AluOpType
    ACT = mybir.ActivationFunctionType

    with tile.TileContext(nc) as tc:
        with (
            tc.tile_pool(name="const", bufs=1) as cpool,
            tc.tile_pool(name="inp", bufs=3) as ipool,
            tc.tile_pool(name="mid", bufs=2) as mpool,
            tc.tile_pool(name="psum", bufs=1, space="PSUM") as ppool,
        ):
            ones_b = cpool.tile([P, 1], BF16)
            nc.vector.memset(ones_b[:], 1.0)
            rate_t = cpool.tile([P, F], BF16)
            nc.sync.dma_start(out=rate_t[:], in_=rate[:])

            accs = []
            for a in range(N_ACC):
                acc = ppool.tile([1, BLK], F32, name=f"acc{a}")
                accs.append(acc)

            for st in range(N_ST):
                p_t = ipool.tile([P, F], F32, name="p_t")
                tb_t = ipool.tile([P, F], BF16, name="tb_t")
                rb_t = ipool.tile([P, F], BF16, name="rb_t")
                nc.sync.dma_start(out=p_t[:], in_=pred_v[st])
                nc.sync.dma_start(out=tb_t[:], in_=tgt_v[st])
                nc.sync.dma_start(out=rb_t[:], in_=rnd_v[st])

                # s = 1 - 2t (fp32), u = pred * s
                s_t = mpool.tile([P, F], F32, name="s_t")
                nc.scalar.activation(s_t[:], tb_t[:], ACT.Copy, bias=1.0, scale=-2.0)
                u_t = mpool.tile([P, F], F32, name="u_t")
                nc.vector.tensor_tensor(u_t[:], p_t[:], s_t[:], TT.mult)

                # bce = softplus(u) = ln(exp(u) + 1), in bf16 for cheap
                # downstream products (exp+ln live in one ACT table set)
                eu_t = mpool.tile([P, F], F32, name="eu_t")
                nc.scalar.activation(eu_t[:], u_t[:], ACT.Exp)
                bce = mpool.tile([P, F], BF16, name="bce")
                nc.scalar.activation(bce[:], eu_t[:], ACT.Ln, bias=1.0)

                # bin masks from bce thresholds
                hard = mpool.tile([P, F], BF16, name="hard")
                nc.vector.tensor_single_scalar(hard[:], bce[:], C_HARD, TT.is_ge)
                easy = mpool.tile([P, F], BF16, name="easy")
                nc.vector.tensor_single_scalar(easy[:], bce[:], C_EASY, TT.is_lt)

                # dropout mask and pass-1 weights
                drop = mpool.tile([P, F], BF16, name="drop")
                nc.vector.tensor_tensor(drop[:], rb_t[:], rate_t[:], TT.is_gt)
                dh = mpool.tile([P, F], BF16, name="dh")
                nc.vector.tensor_tensor(dh[:], drop[:], hard[:], TT.mult)
                w0 = mpool.tile([P, F], BF16, name="w0")
                nc.scalar.activation(w0[:], dh[:], ACT.Copy, bias=1.0, scale=-1.0)

                # products feeding the per-class sums
                tw = mpool.tile([P, F], BF16, name="tw")
                nc.vector.tensor_tensor(tw[:], tb_t[:], w0[:], TT.mult)
                bw = mpool.tile([P, F], BF16, name="bw")
                nc.vector.tensor_tensor(bw[:], bce[:], w0[:], TT.mult)
                bwt = mpool.tile([P, F], BF16, name="bwt")
                nc.vector.tensor_tensor(bwt[:], bw[:], tb_t[:], TT.mult)
                be = mpool.tile([P, F], BF16, name="be")
                nc.vector.tensor_tensor(be[:], bce[:], easy[:], TT.mult)
                bet = mpool.tile([P, F], BF16, name="bet")
                nc.vector.tensor_tensor(bet[:], be[:], tb_t[:], TT.mult)

                rhs_list = [w0, tw, tb_t, bw, bwt, be, bet]
                for a, rhs in enumerate(rhs_list):
                    for b in range(NBLK):
                        m = st * NBLK + b
                        nc.tensor.matmul(
                            accs[a][:, :],
                            ones_b[:, :],
                            rhs[:, b * BLK : (b + 1) * BLK],
                            start=(m == 0),
                            stop=(m == N_ST * NBLK - 1),
                        )

            res = cpool.tile([1, N_ACC * BLK], F32)
            for a in range(N_ACC):
                nc.vector.tensor_copy(res[:, a * BLK : (a + 1) * BLK], accs[a][:, :])
            nc.sync.dma_start(out=out[:], in_=res[:])

    nc.finalize()
    return nc


# ---------------------------------------------------------------------------
# Runner: compile once, execute via PJRT shard_map over 8 axon-tunneled cores.
# Mirrors concourse.bass2jax.run_bass_via_pjrt but caches the jitted callable
# so repeated kernel() calls don't recompile.
# ---------------------------------------------------------------------------
_RUNNER = None


def _make_runner():
    import jax
    from jax.experimental.shard_map import shard_map
    from jax.sharding import Mesh, PartitionSpec

    from concourse import bass2jax

    nc = _build_bass()
    bass2jax.install_neuronx_cc_hook()

    partition_name = (
        nc.partition_id_tensor.name if nc.partition_id_tensor else None
    )
    in_names, out_names, out_avals, zero_outs = [], [], [], []
    for alloc in nc.m.functions[0].allocations:
        if not isinstance(alloc, mybir.MemoryLocationSet):
            continue
        name = alloc.memorylocations[0].name
        if alloc.kind == "ExternalInput":
            if name != partition_name:
                in_names.append(name)
        elif alloc.kind == "ExternalOutput":
            shape = tuple(alloc.tensor_shape)
            dtype = mybir.dt.np(alloc.dtype)
            out_names.append(name)
            out_avals.append(jax.core.ShapedArray(shape, dtype))
            zero_outs.append(np.zeros(shape, dtype))
    n_params = len(in_names)
    n_outs = len(out_avals)
    all_in_names = list(in_names) + list(out_names)
    if partition_name is not None:
        all_in_names = all_in_names + [partition_name]

    def _body(*args):
        operands = list(args)
        if partition_name is not None:
            operands.append(bass2jax.partition_id_tensor())
        outs = bass2jax._bass_exec_p.bind(
            *operands,
            out_avals=tuple(out_avals),
            in_names=tuple(all_in_names),
            out_names=tuple(out_names),
            lowering_input_output_aliases=(),
            sim_require_finite=True,
            sim_require_nnan=True,
            nc=nc,
        )
        return tuple(outs)

    devices = jax.devices()[:N_CORES]
    mesh = Mesh(np.asarray(devices), ("core",))
    in_specs = (PartitionSpec("core"),) * (n_params + n_outs)
    out_specs = (PartitionSpec("core"),) * n_outs
    sharded = jax.jit(
        shard_map(
            _body, mesh=mesh, in_specs=in_specs, out_specs=out_specs, check_rep=False
        ),
        keep_unused=True,
    )
    return {
        "fn": sharded,
        "in_names": in_names,
        "out_names": out_names,
        "zero_outs": zero_outs,
    }


def _get_runner():
    global _RUNNER
    if _RUNNER is None:
        _RUNNER = _make_runner()
    return _RUNNER


def _prep_inputs(pred, target, rand_mat, dropout_rate):
    """Host-side shard/cast: build the concatenated global inputs, keyed by name."""
    pred = np.ascontiguousarray(np.asarray(pred, dtype=np.float32))
    tgt_b = np.asarray(target).astype(ml_dtypes.bfloat16)
    rnd_b = np.asarray(rand_mat).astype(ml_dtypes.bfloat16)
    rate_b = np.asarray(dropout_rate).astype(ml_dtypes.bfloat16)
    # [P, F] pattern: every partition row holds R_ST repeats of the [C] vector
    rate_t = np.tile(rate_b[None, :], (P, R_ST))
    # per-core rate tiles are identical; concat on axis 0 for shard_map
    rate_full = np.tile(rate_t, (N_CORES, 1))
    return {
        "pred": pred,
        "target": tgt_b,
        "rand": rnd_b,
        "rate": rate_full,
    }


def _epilogue(partials):
    """partials: [N_CORES, 1, N_ACC*BLK] fp32 device sums -> scalar loss."""
    flat = partials.reshape(N_CORES, N_ACC, BLK // C, C).astype(np.float64)
    acc = flat.sum(axis=(0, 2))  # [N_ACC, C]
    bc, ps, tsum, A, Bb, Cc, D = acc
    bn = 0.5 * bc
    ns = bc - ps
    pos_gt = (ps >= bn).astype(np.float64)
    neg_gt = (ns > bn).astype(np.float64)
    S = {(1, 1): D, (1, 0): Bb - D, (0, 1): Cc - D, (0, 0): A - Bb - Cc + D}
    cnt = {1: tsum, 0: float(B) - tsum}
    cnt_maj = np.where(pos_gt == 1, cnt[1], cnt[0])
    scale_maj = bn / np.maximum(cnt_maj, 1.0)
    cnt_min = np.where(neg_gt == 1, cnt[1], cnt[0])
    scale_min = (bc - bn) / np.maximum(cnt_min, 1.0)
    total = 0.0
    for t in (0, 1):
        is_maj = t == pos_gt
        is_min = t == neg_gt
        for e in (0, 1):
            f = np.ones(C)
            if e == 1:
                f = np.where(is_maj, 0.0, f)
            f = f * np.where(is_maj, scale_maj, 1.0)
            f = f * np.where(is_min & (cnt_min > 0), scale_min, 1.0)
            total += (f * S[(t, e)]).sum()
    return np.float32(total / (B * C))


def kernel(pred, target, rand_mat, dropout_rate):
    runner = _get_runner()
    named = _prep_inputs(pred, target, rand_mat, dropout_rate)
    ins = [named[n] for n in runner["in_names"]]
    zeros = [
        np.zeros((N_CORES * z.shape[0], *z.shape[1:]), z.dtype)
        for z in runner["zero_outs"]
    ]
    outs = runner["fn"](*ins, *zeros)
    out = np.asarray(outs[0]).reshape(N_CORES, 1, N_ACC * BLK)
    return _epilogue(out)


if __name__ == "__main__":
    rng = np.random.default_rng(0)
    pred = rng.standard_normal((B, C), dtype=np.float32)
    target = rng.integers(0, 2, size=(B, C)).astype(np.float32)
    rand_mat = rng.random((B, C), dtype=np.float32)
    rate = np.ones((C,), dtype=np.float32)
    print("loss:", kernel(pred, target, rand_mat, rate))


# revision 8
# speedup vs baseline: 1597.6574x; 1597.6574x over previous
"""Trainium2 Bass kernel for nn_ComparisonLoss (per-class balanced BCE loss).

Strategy
--------
Data-parallel over the batch across 8 NeuronCores. The whole loss reduces to a
single streaming pass per core that produces 7 per-class sufficient statistics
(each a [40]-vector), followed by a tiny host-side epilogue:

  With t in {0,1}:  u = pred * (1 - 2t)  ==>  bce = softplus(u)
  and |sigmoid(pred) - t| < 0.1  <=>  bce < ln(10/9)   (easy bin)
      |sigmoid(pred) - t| >= 0.9 <=>  bce >= ln(10)    (hard bin)
  (softplus is monotonic, so bin tests become thresholds on bce itself).

Per-class sums accumulated on-device (via ones-vector matmuls into PSUM):
  0: sum(w0)          w0 = 1 - drop*hard   (pass-1 weights)
  1: sum(t*w0)        (pos_sum)
  2: sum(t)
  3: sum(bce*w0)
  4: sum(bce*w0*t)
  5: sum(bce*easy)    (w0 == 1 on easy elements since easy & hard are disjoint)
  6: sum(bce*easy*t)

The majority/minority masking + rescaling of the reference only needs these
sums; the final scalar mean is computed on host from the gathered [7,40]
partials. The 0/1-valued tensors (t, masks, w0) are exact in bf16, so all mask
math runs in bf16 (2x DVE tensor_tensor mode) and the count sums stay
integer-exact in fp32 PSUM, making the majority decisions match the reference
bit-for-bit.
"""

import sys

for _p in ("/opt/trn_rl_repo",):
    if _p not in sys.path:
        sys.path.insert(0, _p)

import numpy as np
import ml_dtypes

import concourse.bacc as bacc
import concourse.tile as tile
from concourse import mybir

# ---- problem constants (hardcoded; kernel.py must be self-contained) ----
B, C = 262144, 40
N_CORES = 8
ROWS_PER_CORE = B // N_CORES          # 32768
P = 128                               # SBUF partitions
ROWS_PER_PART = ROWS_PER_CORE // P    # 256 rows per partition per core
R_ST = 32                             # rows per partition per supertile
N_ST = ROWS_PER_PART // R_ST          # 8 supertiles
F = R_ST * C                          # 1280 free elems per partition per supertile
BLK = 320                             # matmul free width (multiple of C, <=512)
NBLK = F // BLK                       # 4
N_ACC = 7

C_EASY = float(np.log(10.0 / 9.0))    # softplus(-ln 9)
C_HARD = float(np.log(10.0))          # softplus(+ln 9)

F32 = mybir.dt.float32
BF16 = mybir.dt.bfloat16


def _build_bass(iters: int = 1):
    """Build the per-core Bass kernel. iters>1 repeats the full streaming pass
    (re-reading the same DRAM inputs) — used only for loop-delta HW timing."""
    nc = bacc.Bacc("TRN2", target_bir_lowering=False, debug=False)

    pred = nc.dram_tensor("pred", [ROWS_PER_CORE, C], F32, kind="ExternalInput")
    tgt = nc.dram_tensor("target", [ROWS_PER_CORE, C], BF16, kind="ExternalInput")
    rnd = nc.dram_tensor("rand", [ROWS_PER_CORE, C], BF16, kind="ExternalInput")
    rate = nc.dram_tensor("rate", [P, F], BF16, kind="ExternalInput")
    out = nc.dram_tensor("out", [1, N_ACC * BLK], F32, kind="ExternalOutput")

    # row index = st*(P*R_ST) + p*R_ST + r  -> partition p holds contiguous rows
    pred_v = pred.rearrange("(s p r) c -> s p (r c)", s=N_ST, p=P, r=R_ST)
    tgt_v = tgt.rearrange("(s p r) c -> s p (r c)", s=N_ST, p=P, r=R_ST)
    rnd_v = rnd.rearrange("(s p r) c -> s p (r c)", s=N_ST, p=P, r=R_ST)

    TT = mybir.AluOpType
    ACT = mybir.ActivationFunctionType

    with tile.TileContext(nc) as tc:
        with (
            tc.tile_pool(name="const", bufs=1) as cpool,
            tc.tile_pool(name="inp", bufs=3) as ipool,
            tc.tile_pool(name="mid", bufs=2) as mpool,
            tc.tile_pool(name="psum", bufs=1, space="PSUM") as ppool,
        ):
            ones_b = cpool.tile([P, 1], BF16)
            nc.vector.memset(ones_b[:], 1.0)
            rate_t = cpool.tile([P, F], BF16)
            nc.sync.dma_start(out=rate_t[:], in_=rate[:])

            accs = []
            for a in range(N_ACC):
                acc = ppool.tile([1, BLK], F32, name=f"acc{a}")
                accs.append(acc)

            for st_i in range(N_ST * iters):
                st = st_i % N_ST
                p_t = ipool.tile([P, F], F32, name="p_t")
                tb_t = ipool.tile([P, F], BF16, name="tb_t")
                rb_t = ipool.tile([P, F], BF16, name="rb_t")
                nc.sync.dma_start(out=p_t[:], in_=pred_v[st])
                nc.sync.dma_start(out=tb_t[:], in_=tgt_v[st])
                nc.sync.dma_start(out=rb_t[:], in_=rnd_v[st])

                # s = 1 - 2t (fp32), u = pred * s
                s_t = mpool.tile([P, F], F32, name="s_t")
                nc.scalar.activation(s_t[:], tb_t[:], ACT.Copy, bias=1.0, scale=-2.0)
                u_t = mpool.tile([P, F], F32, name="u_t")
                nc.vector.tensor_tensor(u_t[:], p_t[:], s_t[:], TT.mult)

                # bce = softplus(u) = ln(exp(u) + 1), in bf16 for cheap
                # downstream products (exp+ln live in one ACT table set)
                eu_t = mpool.tile([P, F], F32, name="eu_t")
                nc.scalar.activation(eu_t[:], u_t[:], ACT.Exp)
                bce = mpool.tile([P, F], BF16, name="bce")
                nc.scalar.activation(bce[:], eu_t[:], ACT.Ln, bias=1.0)

                # bin masks from bce thresholds
                hard = mpool.tile([P, F], BF16, name="hard")
                nc.vector.tensor_single_scalar(hard[:], bce[:], C_HARD, TT.is_ge)
                easy = mpool.tile([P, F], BF16, name="easy")
                nc.vector.tensor_single_scalar(easy[:], bce[:], C_EASY, TT.is_lt)

                # dropout mask and pass-1 weights
                drop = mpool.tile([P, F], BF16, name="drop")
                nc.vector.tensor_tensor(drop[:], rb_t[:], rate_t[:], TT.is_gt)
                dh = mpool.tile([P, F], BF16, name="dh")
                nc.vector.tensor_tensor(dh[:], drop[:], hard[:], TT.mult)
                w0 = mpool.tile([P, F], BF16, name="w0")
                nc.scalar.activation(w0[:], dh[:], ACT.Copy, bias=1.0, scale=-1.0)

                # products feeding the per-class sums
                tw = mpool.tile([P, F], BF16, name="tw")
                nc.vector.tensor_tensor(tw[:], tb_t[:], w0[:], TT.mult)
                bw = mpool.tile([P, F], BF16, name="bw")
                nc.vector.tensor_tensor(bw[:], bce[:], w0[:], TT.mult)
                bwt = mpool.tile([P, F], BF16, name="bwt")
                nc.vector.tensor_tensor(bwt[:], bw[:], tb_t[:], TT.mult)
                be = mpool.tile([P, F], BF16, name="be")
                nc.vector.tensor_tensor(be[:], bce[:], easy[:], TT.mult)
                bet = mpool.tile([P, F], BF16, name="bet")
                nc.vector.tensor_tensor(bet[:], be[:], tb_t[:], TT.mult)

                rhs_list = [w0, tw, tb_t, bw, bwt, be, bet]
                for a, rhs in enumerate(rhs_list):
                    for b in range(NBLK):
                        m = st_i * NBLK + b
                        nc.tensor.matmul(
                            accs[a][:, :],
                            ones_b[:, :],
                            rhs[:, b * BLK : (b + 1) * BLK],
                            start=(m == 0),
                            stop=(m == N_ST * iters * NBLK - 1),
                        )

            res = cpool.tile([1, N_ACC * BLK], F32)
            for a in range(N_ACC):
                nc.vector.tensor_copy(res[:, a * BLK : (a + 1) * BLK], accs[a][:, :])
            nc.sync.dma_start(out=out[:], in_=res[:])

    nc.finalize()
    return nc


# ---------------------------------------------------------------------------
# Runner: compile once, execute via PJRT shard_map over 8 axon-tunneled cores.
# Mirrors concourse.bass2jax.run_bass_via_pjrt but caches the jitted callable
# so repeated kernel() calls don't recompile.
# ---------------------------------------------------------------------------
_RUNNERS = {}


def _make_runner(iters: int = 1):
    import jax
    from jax.experimental.shard_map import shard_map
    from jax.sharding import Mesh, PartitionSpec

    from concourse import bass2jax

    nc = _build_bass(iters)
    bass2jax.install_neuronx_cc_hook()

    partition_name = (
        nc.partition_id_tensor.name if nc.partition_id_tensor else None
    )
    in_names, out_names, out_avals, zero_outs = [], [], [], []
    for alloc in nc.m.functions[0].allocations:
        if not isinstance(alloc, mybir.MemoryLocationSet):
            continue
        name = alloc.memorylocations[0].name
        if alloc.kind == "ExternalInput":
            if name != partition_name:
                in_names.append(name)
        elif alloc.kind == "ExternalOutput":
            shape = tuple(alloc.tensor_shape)
            dtype = mybir.dt.np(alloc.dtype)
            out_names.append(name)
            out_avals.append(jax.core.ShapedArray(shape, dtype))
            zero_outs.append(np.zeros(shape, dtype))
    n_params = len(in_names)
    n_outs = len(out_avals)
    all_in_names = list(in_names) + list(out_names)
    if partition_name is not None:
        all_in_names = all_in_names + [partition_name]

    def _body(*args):
        operands = list(args)
        if partition_name is not None:
            operands.append(bass2jax.partition_id_tensor())
        outs = bass2jax._bass_exec_p.bind(
            *operands,
            out_avals=tuple(out_avals),
            in_names=tuple(all_in_names),
            out_names=tuple(out_names),
            lowering_input_output_aliases=(),
            sim_require_finite=True,
            sim_require_nnan=True,
            nc=nc,
        )
        return tuple(outs)

    devices = jax.devices()[:N_CORES]
    mesh = Mesh(np.asarray(devices), ("core",))
    in_specs = (PartitionSpec("core"),) * (n_params + n_outs)
    out_specs = (PartitionSpec("core"),) * n_outs
    sharded = jax.jit(
        shard_map(
            _body, mesh=mesh, in_specs=in_specs, out_specs=out_specs, check_rep=False
        ),
        keep_unused=True,
    )
    return {
        "fn": sharded,
        "in_names": in_names,
        "out_names": out_names,
        "zero_outs": zero_outs,
    }


def _get_runner(iters: int = 1):
    if iters not in _RUNNERS:
        _RUNNERS[iters] = _make_runner(iters)
    return _RUNNERS[iters]


def _prep_inputs(pred, target, rand_mat, dropout_rate):
    """Host-side shard/cast: build the concatenated global inputs, keyed by name."""
    pred = np.ascontiguousarray(np.asarray(pred, dtype=np.float32))
    tgt_b = np.asarray(target).astype(ml_dtypes.bfloat16)
    rnd_b = np.asarray(rand_mat).astype(ml_dtypes.bfloat16)
    rate_b = np.asarray(dropout_rate).astype(ml_dtypes.bfloat16)
    # [P, F] pattern: every partition row holds R_ST repeats of the [C] vector
    rate_t = np.tile(rate_b[None, :], (P, R_ST))
    # per-core rate tiles are identical; concat on axis 0 for shard_map
    rate_full = np.tile(rate_t, (N_CORES, 1))
    return {
        "pred": pred,
        "target": tgt_b,
        "rand": rnd_b,
        "rate": rate_full,
    }


def _epilogue(partials):
    """partials: [N_CORES, 1, N_ACC*BLK] fp32 device sums -> scalar loss."""
    flat = partials.reshape(N_CORES, N_ACC, BLK // C, C).astype(np.float64)
    acc = flat.sum(axis=(0, 2))  # [N_ACC, C]
    bc, ps, tsum, A, Bb, Cc, D = acc
    bn = 0.5 * bc
    ns = bc - ps
    pos_gt = (ps >= bn).astype(np.float64)
    neg_gt = (ns > bn).astype(np.float64)
    S = {(1, 1): D, (1, 0): Bb - D, (0, 1): Cc - D, (0, 0): A - Bb - Cc + D}
    cnt = {1: tsum, 0: float(B) - tsum}
    cnt_maj = np.where(pos_gt == 1, cnt[1], cnt[0])
    scale_maj = bn / np.maximum(cnt_maj, 1.0)
    cnt_min = np.where(neg_gt == 1, cnt[1], cnt[0])
    scale_min = (bc - bn) / np.maximum(cnt_min, 1.0)
    total = 0.0
    for t in (0, 1):
        is_maj = t == pos_gt
        is_min = t == neg_gt
        for e in (0, 1):
            f = np.ones(C)
            if e == 1:
                f = np.where(is_maj, 0.0, f)
            f = f * np.where(is_maj, scale_maj, 1.0)
            f = f * np.where(is_min & (cnt_min > 0), scale_min, 1.0)
            total += (f * S[(t, e)]).sum()
    return np.float32(total / (B * C))


def kernel(pred, target, rand_mat, dropout_rate):
    runner = _get_runner()
    named = _prep_inputs(pred, target, rand_mat, dropout_rate)
    ins = [named[n] for n in runner["in_names"]]
    zeros = [
        np.zeros((N_CORES * z.shape[0], *z.shape[1:]), z.dtype)
        for z in runner["zero_outs"]
    ]
    outs = runner["fn"](*ins, *zeros)
    out = np.asarray(outs[0]).reshape(N_CORES, 1, N_ACC * BLK)
    return _epilogue(out)


if __name__ == "__main__":
    rng = np.random.default_rng(0)
    pred = rng.standard_normal((B, C), dtype=np.float32)
    target = rng.integers(0, 2, size=(B, C)).astype(np.float32)
    rand_mat = rng.random((B, C), dtype=np.float32)
    rate = np.ones((C,), dtype=np.float32)
    print("loss:", kernel(pred, target, rand_mat, rate))
